# revision 19
# baseline (speedup 1.0000x reference)
"""GATv2 + BN/MLP actor network on 8 Trainium2 NeuronCores.

Self-contained: host-side edge preprocessing + Bass/Tile kernel + SPMD run.

Strategy:
  - Nodes partitioned across 8 cores by destination (6250 each); per-core dsts
    sorted by in-degree (separately for the L and H sweeps) so fixed-degree
    tile padding is tiny.
  - A node-feature table (|att|-prescaled + a 0.6*(W@att) score column) is
    computed shard-wise and AllGathered (bf16, 256B rows, one filler/special
    row per shard) so every core can dma_gather arbitrary source rows.
  - Edge softmax is dst-local: edges are host-packed into [tiles x 128 dst x D
    slot] gather grids, grouped into uniform-degree batches (one dma_gather +
    ~9 wide DVE ops per batch). Two sweeps (sources from cores 0-4 = "L",
    cores 5-7 = "H") keep dma_gather's int16 indices in range; partial
    softmaxes merge flash-attention style in H-order (the L results are
    re-gathered into H-order during the H sweep).
  - GATv2 score: att.lrelu(u) = 0.6*att.u + 0.4*sum_f att_f |u_f|; the table
    is prescaled by 0.4|att| with positive-att features in columns [0,P) so
    the score is two abs-row-reduces plus the precomputed p-column.
  - MLP head runs feature-major (transposed) so BN is a per-partition scale;
    BN1 stats come from a PE Gram accumulation; stats AllReduce per layer.
"""
import sys

sys.path.insert(0, "/opt/trn_rl_repo")

import numpy as np
import ml_dtypes

N = 50000
E = 800000
OBS = 128
F1 = 80
F2 = 40
AOUT = 32
CORES = 8
SH = N // CORES            # 6250
TPC = 49                   # dst tiles per core
NPAD = TPC * 128           # 6272
LCORES = 5
LCUT = LCORES * SH         # 31250 : positions < LCUT are L-sources
SHF = SH + 1               # AG shard rows incl one filler (= special) row
GTR = CORES * SHF          # 50008 gather-table rows
LAPR = LCORES * SHF        # 31255 : L region = rows [0, 31255)
HAPB = LAPR                # H region base row
LPAD = SH                  # L pad idx -> core 0's filler row
HPAD = GTR - 1 - HAPB      # H pad idx -> core 7's filler row (18752)
RW = 128                   # table row width (bf16 -> 256B)
EPS_BN = 1e-5
NEG_BIG = -30000.0


# ---------------------------------------------------------------- host prep
def _pack_idx(flat):
    """int16 flat index list -> [128, ceil(n/16)] wrapped+replicated layout."""
    n = len(flat)
    assert n % 16 == 0
    w = flat.reshape(n // 16, 16).T.astype(np.int16)  # [16, n/16]
    return np.tile(w, (8, 1))


def _csr_slots(crow, nrows):
    """For edges sorted by crow: slot index of each edge within its row."""
    order = np.argsort(crow, kind="stable")
    sr = crow[order]
    counts = np.bincount(sr, minlength=nrows)
    starts = np.zeros(nrows + 1, np.int64)
    np.cumsum(counts, out=starts[1:])
    slot = np.arange(len(order), dtype=np.int64) - starts[sr]
    return order, sr, slot


def _grid_idx(crow, slot, idxval, Dts, pad):
    """Scatter edge idxvals into the per-tile slot-major gather list."""
    offs = np.zeros(TPC + 1, np.int64)
    np.cumsum(np.asarray(Dts, np.int64) * 128, out=offs[1:])
    total = int(offs[-1])
    out = np.full(total, pad, np.int64)
    t = crow // 128
    p = crow % 128
    pos = offs[t] + slot * 128 + p
    out[pos] = idxval
    return out


def _prep(x, edge_index, W_gat, att, bias_gat, g1, b1, g2, b2, g3, b3, W1, W2, W3):
    # bias_gat cancels inside train-mode BatchNorm (shifts h1 and its batch
    # mean equally), so it is accepted and ignored.
    # self-loops are handled on-device from the local table (no gather), so
    # the edge lists carry only the real edges.
    src = np.asarray(edge_index[0]).astype(np.int64)
    dst = np.asarray(edge_index[1]).astype(np.int64)

    isL = (src // SH) < LCORES
    dL = np.bincount(dst[isL], minlength=N)
    dH = np.bincount(dst[~isL], minlength=N)

    perm = np.empty(N, np.int64)           # perm[pos] = orig id (canonical/L)
    pos_of = np.empty(N, np.int64)         # pos_of[orig] = pos
    canon_row = np.empty(N, np.int64)      # orig -> local canonical row
    horder_row = np.empty(N, np.int64)     # orig -> local H-order row
    hperm = np.empty((CORES, NPAD), np.int64)   # H-row -> canonical row
    outperm = np.empty(N, np.int64)        # output row (c, j H-order) -> orig
    for c in range(CORES):
        lo = c * SH
        ids = lo + np.argsort(-dL[lo:lo + SH], kind="stable")
        perm[lo:lo + SH] = ids
        pos_of[ids] = np.arange(lo, lo + SH)
        canon_row[ids] = np.arange(SH)
        hids = lo + np.argsort(-dH[lo:lo + SH], kind="stable")
        horder_row[hids] = np.arange(SH)
        hp = np.arange(NPAD, dtype=np.int64)
        hp[:SH] = canon_row[hids]
        hperm[c] = hp
        outperm[lo:lo + SH] = hids

    # uniform per-tile degrees across cores
    DL = np.ones(TPC, np.int64)
    DH = np.ones(TPC, np.int64)
    for c in range(CORES):
        lo = c * SH
        sdl = np.zeros(NPAD, np.int64)
        sdl[:SH] = -np.sort(-dL[lo:lo + SH])
        sdh = np.zeros(NPAD, np.int64)
        sdh[:SH] = -np.sort(-dH[lo:lo + SH])
        DL = np.maximum(DL, sdl.reshape(TPC, 128).max(1))
        DH = np.maximum(DH, sdh.reshape(TPC, 128).max(1))
    DL = [int(v) for v in DL]
    DH = [int(v) for v in DH]

    def mk_groups(Ds, cap=36, maxt=8, tol=1):
        gs = []
        t = 0
        while t < TPC:
            Dg = max(Ds[t], 1)
            take = 1
            while (t + take < TPC and take < maxt
                   and Dg * (take + 1) <= cap
                   and Dg - max(Ds[t + take], 1) <= tol):
                take += 1
            gs.append((t, t + take, Dg))
            t += take
        return gs

    GL = mk_groups(DL)
    GH = mk_groups(DH)
    DL = [Dg for (a, b, Dg) in GL for _ in range(b - a)]
    DH = [Dg for (a, b, Dg) in GH for _ in range(b - a)]

    lidx, hidx = [], []
    for c in range(CORES):
        mc = (dst // SH) == c
        for is_l, Dts, pad, out in ((True, DL, LPAD, lidx), (False, DH, HPAD, hidx)):
            m = mc & (isL if is_l else ~isL)
            es, ed = src[m], dst[m]
            crow = canon_row[ed] if is_l else horder_row[ed]
            order, sr, slot = _csr_slots(crow, SH)
            ps_ = pos_of[es][order]
            rows = ps_ + ps_ // SH                       # skip filler rows
            idxval = rows if is_l else rows - HAPB
            out.append(_pack_idx(_grid_idx(sr, slot, idxval, Dts, pad)))

    # feature permutation: positive-att first
    att = np.asarray(att, np.float64)
    neg = att < 0
    fperm = np.argsort(neg, kind="stable")
    P = int((~neg).sum())
    absatt = np.maximum(0.4 * np.abs(att[fperm]), 1e-12)

    Wg = np.asarray(W_gat, np.float64)
    wtab = np.concatenate(
        [Wg[:, fperm] * absatt[None, :], 0.6 * (Wg @ att)[:, None]], axis=1
    ).astype(ml_dtypes.bfloat16)                          # [128, 81]
    invatt = np.tile((1.0 / absatt)[None, :], (128, 1)).astype(np.float32)

    spec = np.zeros((1, RW), ml_dtypes.bfloat16)
    spec[0, F1] = NEG_BIG  # p-column; all else zero

    bnp = np.stack([np.asarray(g1)[fperm], np.asarray(b1)[fperm],
                    np.asarray(g2), np.asarray(b2)], axis=1).astype(np.float32)
    bn3 = np.stack([np.asarray(g3), np.asarray(b3)], axis=1).astype(np.float32)

    shared = {
        "wtab": wtab,
        "invatt": invatt,
        "specrow": spec,
        "w1": np.asarray(W1)[fperm, :].astype(ml_dtypes.bfloat16),
        "w2": np.asarray(W2).astype(ml_dtypes.bfloat16),
        "w3": np.asarray(W3).astype(ml_dtypes.bfloat16),
        "bnp": bnp,
        "bn3": bn3,
    }
    x = np.asarray(x, np.float32)
    in_maps = []
    for c in range(CORES):
        im = dict(shared)
        im["xT"] = np.ascontiguousarray(x[perm[c * SH:(c + 1) * SH]].T).astype(ml_dtypes.bfloat16)
        im["xTH"] = np.ascontiguousarray(x[outperm[c * SH:(c + 1) * SH]].T).astype(ml_dtypes.bfloat16)
        im["lidx"] = lidx[c]
        im["hidx"] = hidx[c]
        im["lalidx"] = _pack_idx(hperm[c])
        in_maps.append(im)
    return in_maps, outperm, P, GL, GH, DL, DH


# ---------------------------------------------------------------- device build
def _build(P, GL, GH, DL, DH):
    import concourse.bacc as bacc
    import concourse.mybir as mybir
    import concourse.tile as tile
    from concourse.masks import make_identity

    dt = mybir.dt
    BF = dt.bfloat16
    FP = dt.float32
    ADD = mybir.AluOpType.add
    SUB = mybir.AluOpType.subtract
    MUL = mybir.AluOpType.mult
    MAX = mybir.AluOpType.max
    AF = mybir.ActivationFunctionType
    X = mybir.AxisListType.X

    LW = sum(DL) * 8
    HW = sum(DH) * 8
    GW = NPAD // 16

    nc = bacc.Bacc("TRN2", target_bir_lowering=False, debug=False,
                   num_devices=CORES, num_swdge_queues=4)
    p_xT = nc.declare_dram_parameter("xT", [OBS, SH], BF, isOutput=False)
    p_xTH = nc.declare_dram_parameter("xTH", [OBS, SH], BF, isOutput=False)
    p_lidx = nc.declare_dram_parameter("lidx", [128, LW], dt.int16, isOutput=False)
    p_hidx = nc.declare_dram_parameter("hidx", [128, HW], dt.int16, isOutput=False)
    p_lalidx = nc.declare_dram_parameter("lalidx", [128, GW], dt.int16, isOutput=False)
    p_wtab = nc.declare_dram_parameter("wtab", [OBS, F1 + 1], BF, isOutput=False)
    p_invatt = nc.declare_dram_parameter("invatt", [128, F1], FP, isOutput=False)
    p_spec = nc.declare_dram_parameter("specrow", [1, RW], BF, isOutput=False)
    p_w1 = nc.declare_dram_parameter("w1", [F1, F1], BF, isOutput=False)
    p_w2 = nc.declare_dram_parameter("w2", [F1, F2], BF, isOutput=False)
    p_w3 = nc.declare_dram_parameter("w3", [F2, AOUT], BF, isOutput=False)
    p_bnp = nc.declare_dram_parameter("bnp", [F1, 4], FP, isOutput=False)
    p_bn3 = nc.declare_dram_parameter("bn3", [F2, 2], FP, isOutput=False)
    p_out = nc.declare_dram_parameter("out", [AOUT, SH], FP, isOutput=True)

    RG = [list(range(CORES))]

    with tile.TileContext(nc) as tc:
        with (
            tc.tile_pool(name="persist", bufs=1) as pp,
            tc.tile_pool(name="dram", bufs=1, space="DRAM") as dp,
            tc.tile_pool(name="psum", bufs=2, space="PSUM") as psp,
            tc.tile_pool(name="work", bufs=2) as wp,
            tc.tile_pool(name="small", bufs=3) as sp,
        ):
            gtable = dp.tile([GTR, RW], BF, addr_space="Shared", name="gtable")
            glocald = dp.tile([NPAD, RW], BF, name="glocald")
            lbufd = dp.tile([NPAD, RW], FP, name="lbufd")

            # ---- preload constants / indices
            wtab_sb = pp.tile([OBS, F1 + 1], BF, name="wtab_sb")
            nc.sync.dma_start(out=wtab_sb[:], in_=p_wtab[:, :])
            invatt_sb = pp.tile([128, F1], FP, name="invatt_sb")
            nc.sync.dma_start(out=invatt_sb[:], in_=p_invatt[:, :])
            spec_sb = pp.tile([1, RW], BF, name="spec_sb")
            nc.sync.dma_start(out=spec_sb[:], in_=p_spec[:, :])
            lidx_sb = pp.tile([128, LW], dt.int16, name="lidx_sb")
            nc.sync.dma_start(out=lidx_sb[:], in_=p_lidx[:, :])
            hidx_sb = pp.tile([128, HW], dt.int16, name="hidx_sb")
            nc.sync.dma_start(out=hidx_sb[:], in_=p_hidx[:, :])
            lalidx_sb = pp.tile([128, GW], dt.int16, name="lalidx_sb")
            nc.sync.dma_start(out=lalidx_sb[:], in_=p_lalidx[:, :])
            ident = pp.tile([128, 128], FP, name="ident")
            make_identity(nc, ident[:])

            glocal = pp.tile([128, TPC, RW], BF, name="glocal")
            nc.vector.memset(glocal[:], 0.0)
            glocalh = pp.tile([128, TPC, RW], BF, name="glocalh")
            nc.vector.memset(glocalh[:], 0.0)

            # ---- phase T: node feature tables = x @ wtab (prescaled)
            tscope = tc.tile_pool(name="phT", bufs=1)
            tp_ = tscope.__enter__()
            xT_sb = tp_.tile([OBS, SH], BF, name="xT_sb")
            nc.sync.dma_start(out=xT_sb[:], in_=p_xT[:, :])
            for t in range(TPC):
                n0 = t * 128
                m = min(128, SH - n0)
                if m <= 0:
                    break
                pt = psp.tile([128, F1 + 1], FP, name="pt", tag="pt")
                nc.tensor.matmul(pt[:m, :], lhsT=xT_sb[:, n0:n0 + m],
                                 rhs=wtab_sb[:], start=True, stop=True)
                nc.scalar.activation(glocal[:m, t, 0:F1 + 1], pt[:m, :], AF.Copy)
                # spill this tile's table rows now so the store overlaps the
                # remaining matmuls instead of serializing before AllGather
                nc.sync.dma_start(out=glocald[n0:n0 + m, :],
                                  in_=glocal[:m, t, :])
            nc.sync.dma_start(out=glocald[SH:SH + 1, :], in_=spec_sb[:])
            nc.gpsimd.collective_compute(
                "AllGather", mybir.AluOpType.bypass, replica_groups=RG,
                ins=[glocald[0:SHF, :].opt()],
                outs=[gtable[0:GTR, :].opt()],
            )
            zfill = tp_.tile([128, TPC, 47], FP, name="zfill")
            nc.vector.memset(zfill[:], 0.0)
            nc.sync.dma_start(
                out=lbufd[:, 81:128].rearrange("(t p) w -> p t w", p=128),
                in_=zfill[:])
            tscope.__exit__(None, None, None)

            lres = pp.tile([128, TPC, 88], FP, name="lres")
            hres = pp.tile([128, TPC, 88], FP, name="hres")
            escope = tc.tile_pool(name="ewk", bufs=2)
            ep_ = escope.__enter__()

            # ---- edge sweeps (grouped: one gather + wide ops per group)
            # Desc-gen for queue q runs on Q7 core pair (2q, 2q+1); spreading
            # gathers over all 4 SWDGE queues parallelizes descriptor
            # generation, which is the serial bottleneck at 1 queue.
            qrr = [0]

            def sweep(groups, idx_sb, table_lo, table_hi, gdst, res):
                off8 = 0
                for (t0, t1, D) in groups:
                    Tg = t1 - t0
                    GD = Tg * D
                    gsrc = ep_.tile([128, Tg, D, RW], BF, name="gsrc",
                                    tag="gsrc", bufs=4)
                    nc.gpsimd.dma_gather(
                        out_ap=gsrc[:].rearrange("p t d w -> p (t d) w"),
                        in_ap=gtable[table_lo:table_hi, :],
                        idxs_ap=idx_sb[:, off8:off8 + GD * 8],
                        num_idxs=GD * 128, num_idxs_reg=GD * 128,
                        elem_size=RW, elem_step=RW,
                        single_packet=(GD * 128 <= 1024),
                        queue_num=qrr[0] % 4)
                    qrr[0] += 1
                    off8 += GD * 8
                    # u = h_src + h_dst; the ONLY reader of gsrc (aggregation
                    # uses u too: sum ex*u = sum ex*gsrc + den*gdst, corrected
                    # in the merge), so the gather buffer frees immediately.
                    u = ep_.tile([128, Tg, D, 84], BF, name="u", tag="u", bufs=3)
                    nc.vector.tensor_tensor(
                        out=u[:], in0=gsrc[:, :, :, 0:84],
                        in1=gdst[:, t0:t1, None, 0:84]
                            .to_broadcast([128, Tg, D, 84]),
                        op=ADD)
                    ework = ep_.tile([128, Tg, D, 2], FP, name="ework",
                                     tag="ework", bufs=3)
                    eet = ep_.tile([128, Tg, D], FP, name="eet", tag="eet", bufs=3)
                    rp = ework[:, :, :, 0].opt()
                    rn = ework[:, :, :, 1].opt()
                    ee = eet[:].opt()
                    if P > 0:
                        nc.vector.tensor_reduce(
                            out=rp, in_=u[:, :, :, 0:P], axis=X, op=ADD,
                            apply_absolute_value=True)
                    else:
                        nc.vector.memset(rp, 0.0)
                    if P < F1:
                        nc.vector.tensor_reduce(
                            out=rn, in_=u[:, :, :, P:F1], axis=X, op=ADD,
                            apply_absolute_value=True, negate=True)
                    else:
                        nc.vector.memset(rn, 0.0)
                    nc.vector.tensor_tensor(out=ee, in0=rp, in1=rn, op=ADD)
                    nc.vector.tensor_tensor(out=ee, in0=ee,
                                            in1=u[:, :, :, F1].opt(), op=ADD)
                    # unshifted exp (softmax is shift-invariant; scores are
                    # O(10) so fp32/bf16 exp cannot overflow; pad rows carry
                    # -30000 and underflow to exactly 0)
                    ex = ep_.tile([128, Tg, D], BF, name="ex", tag="ex", bufs=3)
                    nc.scalar.activation(ex[:], eet[:], AF.Exp)
                    nc.vector.tensor_reduce(out=res[:, t0:t1, 80].opt(),
                                            in_=ex[:], axis=X, op=ADD)
                    # weighted aggregation (of u, not gsrc — see above);
                    # sum over d by pairwise folding (bf16 TT runs in 2x DVE
                    # mode; the strided tensor_reduce ran at ~0.45 elem/cyc).
                    # The last fold writes straight into fp32 res.
                    if D == 1:
                        nc.vector.tensor_tensor(
                            out=res[:, t0:t1, 0:F1].opt(),
                            in0=u[:, :, 0, 0:F1],
                            in1=ex[:, :, 0, None].to_broadcast([128, Tg, F1]),
                            op=MUL)
                    else:
                        ag = ep_.tile([128, Tg, D, F1], BF, name="ag",
                                      tag="ag", bufs=3)
                        nc.vector.tensor_tensor(
                            out=ag[:], in0=u[:, :, :, 0:F1],
                            in1=ex[:, :, :, None]
                                .to_broadcast([128, Tg, D, F1]),
                            op=MUL)
                        dd = D
                        while dd > 2:
                            h = dd // 2
                            nc.vector.tensor_tensor(
                                out=ag[:, :, 0:h, :], in0=ag[:, :, 0:h, :],
                                in1=ag[:, :, dd - h:dd, :], op=ADD)
                            dd -= h
                        nc.vector.tensor_tensor(
                            out=res[:, t0:t1, 0:F1].opt(),
                            in0=ag[:, :, 0, :], in1=ag[:, :, 1, :], op=ADD)

            sweep(GL, lidx_sb, 0, LAPR, glocal, lres)
            # H-order table (needed once the H sweep starts; fills PE idle)
            xTH_sb = ep_.tile([OBS, SH], BF, name="xTH_sb", bufs=1)
            nc.sync.dma_start(out=xTH_sb[:], in_=p_xTH[:, :])
            for t in range(TPC):
                n0 = t * 128
                m = min(128, SH - n0)
                if m <= 0:
                    break
                pth = psp.tile([128, F1 + 1], FP, name="pth", tag="pt")
                nc.tensor.matmul(pth[:m, :], lhsT=xTH_sb[:, n0:n0 + m],
                                 rhs=wtab_sb[:], start=True, stop=True)
                nc.scalar.activation(glocalh[:m, t, 0:F1 + 1], pth[:m, :], AF.Copy)
            # L results -> HBM -> regather into H-order (overlaps the H sweep)
            nc.sync.dma_start(
                out=lbufd[:, 0:81].rearrange("(t p) w -> p t w", p=128),
                in_=lres[:, :, 0:81])
            lal = pp.tile([128, TPC, RW], FP, name="lal")
            nc.gpsimd.dma_gather(
                out_ap=lal[:], in_ap=lbufd[:, :], idxs_ap=lalidx_sb[:],
                num_idxs=NPAD, num_idxs_reg=NPAD, elem_size=RW, elem_step=RW,
                single_packet=False, queue_num=qrr[0] % 4)
            qrr[0] += 1
            # ---- self-loop partial: ee_s = 2*(rp - rn + pcol) from glocalh
            # (needs only the H-order table; runs in the L->H transition)
            sres = pp.tile([128, TPC, 4], FP, name="sres")
            if P > 0:
                nc.vector.tensor_reduce(
                    out=sres[:, :, 0].opt(), in_=glocalh[:, :, 0:P], axis=X,
                    op=ADD, apply_absolute_value=True)
            else:
                nc.vector.memset(sres[:, :, 0].opt(), 0.0)
            if P < F1:
                nc.vector.tensor_reduce(
                    out=sres[:, :, 1].opt(), in_=glocalh[:, :, P:F1], axis=X,
                    op=ADD, apply_absolute_value=True, negate=True)
            else:
                nc.vector.memset(sres[:, :, 1].opt(), 0.0)
            nc.vector.tensor_copy(out=sres[:, :, 2].opt(),
                                  in_=glocalh[:, :, F1].opt())
            nc.vector.tensor_tensor(out=sres[:, :, 3].opt(),
                                    in0=sres[:, :, 0].opt(),
                                    in1=sres[:, :, 1].opt(), op=ADD)
            nc.vector.tensor_tensor(out=sres[:, :, 3].opt(),
                                    in0=sres[:, :, 3].opt(),
                                    in1=sres[:, :, 2].opt(), op=ADD)
            nc.vector.tensor_scalar_mul(sres[:, :, 3].opt(),
                                        sres[:, :, 3].opt(), 2.0)

            sweep(GH, hidx_sb, HAPB, GTR, glocalh, hres)
            escope.__exit__(None, None, None)
            mlpscope = tc.tile_pool(name="mlp", bufs=1)
            mp_ = mlpscope.__enter__()

            # ---- merge the three partials (H-order, per-half so the
            # first half overlaps the tail of the H sweep). No max shifting
            # (unshifted softmax), and the partial sums carry an extra
            # den*gdst from aggregating u; (num*rec - gdst)*invatt fixes it.
            mrg = pp.tile([128, TPC, 8], FP, name="mrg")
            h1 = pp.tile([128, TPC, F1 + 1], FP, name="h1")
            nc.vector.memset(h1[:], 1.0)   # col 80 stays 1 for the stats matmul

            def merge_half(a, b):
                W = b - a
                ws = mrg[:, a:b, 1].opt()
                ws2 = mrg[:, a:b, 2].opt()
                den = mrg[:, a:b, 3].opt()
                rec = mrg[:, a:b, 4].opt()
                ees = sres[:, a:b, 3].opt()
                nc.scalar.activation(ws, ees, AF.Exp)
                nc.vector.tensor_tensor(out=den, in0=lal[:, a:b, 80].opt(),
                                        in1=hres[:, a:b, 80].opt(), op=ADD)
                nc.vector.tensor_tensor(out=den, in0=den, in1=ws, op=ADD)
                nc.vector.tensor_scalar_add(den, den, 1e-16)
                nc.vector.reciprocal(rec, den)
                # self-loop u = 2*gdst
                nc.vector.tensor_scalar_mul(ws2, ws, 2.0)
                nc.vector.tensor_tensor(
                    out=h1[:, a:b, 0:F1], in0=glocalh[:, a:b, 0:F1],
                    in1=mrg[:, a:b, 2:3].to_broadcast([128, W, F1]), op=MUL)
                nc.vector.tensor_tensor(out=h1[:, a:b, 0:F1],
                                        in0=h1[:, a:b, 0:F1],
                                        in1=hres[:, a:b, 0:F1], op=ADD)
                nc.vector.tensor_tensor(out=h1[:, a:b, 0:F1],
                                        in0=h1[:, a:b, 0:F1],
                                        in1=lal[:, a:b, 0:F1], op=ADD)
                nc.vector.tensor_tensor(
                    out=h1[:, a:b, 0:F1], in0=h1[:, a:b, 0:F1],
                    in1=mrg[:, a:b, 4:5].to_broadcast([128, W, F1]), op=MUL)
                nc.vector.tensor_tensor(
                    out=h1[:, a:b, 0:F1], in0=h1[:, a:b, 0:F1],
                    in1=glocalh[:, a:b, 0:F1], op=SUB)
                nc.vector.tensor_tensor(
                    out=h1[:, a:b, 0:F1], in0=h1[:, a:b, 0:F1],
                    in1=invatt_sb[:, None, :].to_broadcast([128, W, F1]), op=MUL)

            # gram (BN1 stats) matmuls interleave with the merge halves so
            # the PE accumulation runs as soon as each h1 span is merged
            gram = psp.tile([F1, F1 + 1], FP, name="gram", tag="gram", bufs=1)
            cuts = [0, 18, 34, TPC] if TPC > 34 else [0, TPC]
            for ci in range(len(cuts) - 1):
                merge_half(cuts[ci], cuts[ci + 1])
                for t in range(cuts[ci], cuts[ci + 1]):
                    nc.tensor.matmul(gram[:], lhsT=h1[:, t, 0:F1],
                                     rhs=h1[:, t, :], start=(t == 0),
                                     stop=(t == TPC - 1))

            # ---- weights for the MLP head
            bnp_sb = pp.tile([F1, 4], FP, name="bnp_sb")
            nc.sync.dma_start(out=bnp_sb[:], in_=p_bnp[:, :])
            bn3_sb = pp.tile([F2, 2], FP, name="bn3_sb")
            nc.sync.dma_start(out=bn3_sb[:], in_=p_bn3[:, :])
            w1_sb = pp.tile([F1, F1], BF, name="w1_sb")
            nc.sync.dma_start(out=w1_sb[:], in_=p_w1[:, :])
            w2_sb = pp.tile([F1, F2], BF, name="w2_sb")
            nc.sync.dma_start(out=w2_sb[:], in_=p_w2[:, :])
            w3_sb = pp.tile([F2, AOUT], BF, name="w3_sb")
            nc.sync.dma_start(out=w3_sb[:], in_=p_w3[:, :])

            # ---- BN1 stats from the Gram accumulation
            stat1 = sp.tile([F1, 2], FP, name="stat1", tag="stat1")
            diag = sp.tile([F1, F1], FP, name="diag", tag="diag")
            nc.vector.tensor_tensor(out=diag[:], in0=gram[:, 0:F1],
                                    in1=ident[0:F1, 0:F1], op=MUL)
            nc.vector.tensor_reduce(out=stat1[:, 1:2], in_=diag[:], axis=X, op=ADD)
            nc.scalar.activation(stat1[:, 0:1], gram[:, F1:F1 + 1], AF.Copy)

            # ---- transpose h1 to feature-major [F1, NPAD]
            h1T = mp_.tile([F1, NPAD], BF, name="h1T", tag="hT", bufs=2)
            for t in range(TPC):
                ps = psp.tile([F1, 128], FP, name="ps", tag="mm")
                nc.tensor.transpose(ps[:], h1[:, t, 0:F1], ident[:])
                nc.scalar.activation(h1T[:, t * 128:(t + 1) * 128], ps[:], AF.Copy)

            # ---- BN folded into the next matmul: W'=diag(sc)W, b'=W^T bb,
            # so layer k+1 is act(W'^T h + b') and no [F, NPAD] BN pass runs
            def bn_fold(i, F, FOUT, stat, g_ap, b_ap, w_sb):
                arin = dp.tile([F, 2], FP, name=f"arin{i}", tag=f"arin{i}")
                arout = dp.tile([CORES * F, 2], FP, name=f"arout{i}",
                                tag=f"arout{i}", addr_space="Shared")
                nc.sync.dma_start(out=arin[:], in_=stat[:])
                nc.gpsimd.collective_compute(
                    "AllGather", mybir.AluOpType.bypass, replica_groups=RG,
                    ins=[arin[:].opt()], outs=[arout[:].opt()])
                ga = sp.tile([F, CORES, 2], FP, name=f"ga{i}", tag=f"ga{i}")
                nc.sync.dma_start(
                    out=ga[:],
                    in_=arout[:].rearrange("(g f) s -> f g s", g=CORES))
                for half in (4, 2, 1):
                    nc.vector.tensor_tensor(
                        out=ga[:, 0:half, :], in0=ga[:, 0:half, :],
                        in1=ga[:, half:2 * half, :], op=ADD)
                gs = sp.tile([F, 6], FP, name=f"gs{i}", tag=f"gs{i}")
                nc.vector.tensor_copy(out=gs[:, 0:2], in_=ga[:, 0, :])
                mu = gs[:, 2:3]
                var = gs[:, 3:4]
                sc = gs[:, 4:5]
                bb = gs[:, 5:6]
                nc.vector.tensor_scalar_mul(mu, gs[:, 0:1], 1.0 / N)
                nc.vector.tensor_scalar_mul(var, gs[:, 1:2], 1.0 / N)
                nc.vector.tensor_tensor(out=sc, in0=mu, in1=mu, op=MUL)
                nc.vector.tensor_tensor(out=var, in0=var, in1=sc, op=SUB)
                nc.vector.tensor_scalar_add(var, var, EPS_BN)
                nc.scalar.activation(sc, var, AF.Sqrt, bias=0.0)
                nc.vector.reciprocal(sc, sc)
                nc.vector.tensor_tensor(out=sc, in0=sc, in1=g_ap, op=MUL)
                nc.vector.tensor_tensor(out=bb, in0=mu, in1=sc, op=MUL)
                nc.vector.tensor_tensor(out=bb, in0=b_ap, in1=bb, op=SUB)
                wsc = mp_.tile([F, FOUT], BF, name=f"wsc{i}", tag=f"wsc{i}")
                nc.scalar.activation(wsc[:], w_sb[:], AF.Identity,
                                     bias=0.0, scale=sc)
                bbb = sp.tile([F, 1], BF, name=f"bbb{i}", tag=f"bbb{i}")
                nc.vector.tensor_copy(out=bbb[:], in_=bb)
                pb = psp.tile([FOUT, 1], FP, name=f"pb{i}", tag="pb")
                nc.tensor.matmul(pb[:], lhsT=w_sb[:], rhs=bbb[:],
                                 start=True, stop=True)
                bv = sp.tile([FOUT, 1], FP, name=f"bv{i}", tag=f"bv{i}")
                nc.scalar.activation(bv[:], pb[:], AF.Copy)
                return wsc, bv

            def stats_dve(i, F, hT):
                stat = sp.tile([F, 2], FP, name=f"stat{i}", tag=f"stat{i}")
                nc.vector.tensor_reduce(out=stat[:, 0:1], in_=hT[:, 0:SH],
                                        axis=X, op=ADD)
                nch = (SH + 511) // 512
                sq = wp.tile([F, 512], FP, name=f"sq{i}", tag="sq")
                pstat = sp.tile([F, nch], FP, name=f"pstat{i}", tag=f"pstat{i}")
                for ci, c0 in enumerate(range(0, SH, 512)):
                    cw = min(512, SH - c0)
                    nc.vector.scalar_tensor_tensor(
                        out=sq[:, 0:cw], in0=hT[:, c0:c0 + cw], scalar=1.0,
                        in1=hT[:, c0:c0 + cw], op0=MUL, op1=MUL,
                        accum_out=pstat[:, ci:ci + 1])
                nc.vector.tensor_reduce(out=stat[:, 1:2], in_=pstat[:],
                                        axis=X, op=ADD)
                return stat

            w1f, bv1 = bn_fold(1, F1, F1, stat1,
                               bnp_sb[:, 0:1], bnp_sb[:, 1:2], w1_sb)
            h2T = mp_.tile([F1, NPAD], BF, name="h2T", tag="hT", bufs=2)
            for c0 in range(0, NPAD, 512):
                cw = min(512, NPAD - c0)
                pm = psp.tile([F1, 512], FP, name="pm1", tag="mm")
                nc.tensor.matmul(pm[:, 0:cw], lhsT=w1f[:], rhs=h1T[:, c0:c0 + cw],
                                 start=True, stop=True)
                nc.scalar.activation(h2T[:, c0:c0 + cw], pm[:, 0:cw], AF.Relu,
                                     bias=bv1[:])

            st2 = stats_dve(2, F1, h2T)
            w2f, bv2 = bn_fold(2, F1, F2, st2,
                               bnp_sb[:, 2:3], bnp_sb[:, 3:4], w2_sb)
            h3T = mp_.tile([F2, NPAD], BF, name="h3T", tag="hT", bufs=2)
            for c0 in range(0, NPAD, 512):
                cw = min(512, NPAD - c0)
                pm = psp.tile([F2, 512], FP, name="pm2", tag="mm")
                nc.tensor.matmul(pm[:, 0:cw], lhsT=w2f[:], rhs=h2T[:, c0:c0 + cw],
                                 start=True, stop=True)
                nc.scalar.activation(h3T[:, c0:c0 + cw], pm[:, 0:cw], AF.Relu,
                                     bias=bv2[:])

            st3 = stats_dve(3, F2, h3T)
            w3f, bv3 = bn_fold(3, F2, AOUT, st3,
                               bn3_sb[:, 0:1], bn3_sb[:, 1:2], w3_sb)
            for c0 in range(0, SH, 512):
                cw = min(512, SH - c0)
                pm = psp.tile([AOUT, 512], FP, name="pm3", tag="mm")
                nc.tensor.matmul(pm[:, 0:cw], lhsT=w3f[:], rhs=h3T[:, c0:c0 + cw],
                                 start=True, stop=True)
                oc = mp_.tile([AOUT, 512], FP, name="oc", tag="oc", bufs=3)
                nc.scalar.activation(oc[:, 0:cw], pm[:, 0:cw], AF.Sigmoid,
                                     bias=bv3[:])
                nc.vector.tensor_scalar_max(oc[:, 0:cw], oc[:, 0:cw], 0.5)
                nc.sync.dma_start(out=p_out[:, c0:c0 + cw], in_=oc[:, 0:cw])
            mlpscope.__exit__(None, None, None)

    nc.compile()
    return nc


# ---------------------------------------------------------------- entry point
def kernel(**inputs):
    from concourse import bass_utils

    in_maps, outperm, P, GL, GH, DL, DH = _prep(**inputs)
    nc = _build(P, GL, GH, DL, DH)
    res = bass_utils.run_bass_kernel_spmd(
        nc, in_maps, core_ids=list(range(CORES)))
    out = np.empty((N, AOUT), np.float32)
    for c in range(CORES):
        out[outperm[c * SH:(c + 1) * SH]] = res.results[c]["out"].T
    return out


if __name__ == "__main__":
    import reference
    ins = {k: np.asarray(v) for k, v in reference.setup_inputs().items()}
    got = kernel(**ins)
    print("out", got.shape, got.dtype)

